# revision 2
# baseline (speedup 1.0000x reference)
"""MixerDiffAttention TRN2 kernel v2 (8-core TP over head pairs).

Per-core math (head-pair i: q/k heads i,i+8; v head i):
  q,k,v projections as fp8e4m3 DoubleRow matmuls with hi/lo error
  compensation: host splits x*16 and W*128 into fp8 hi + fp8 residual;
  q = xh@Wh + xl@Wh + xh@Wl (0.75x fp32r cost, ~1e-3 accurate).
  rms_norm is scale-invariant so q/k need no descale; the v descale
  (1/2048) is folded into the l-sum ones constant (2048.0).
  Scores/PV/p in bf16. Per-head rms normalization folds into the
  transpose: one bf16 matmul against diag(r) built by DVE from a bf16
  identity (qT = yn.T @ diag(r)). exp applies the 1/sqrt(256) scale.

Schedule: single pass per batch - each token pair projects k/v/q then
runs attention for that q-chunk (next pair's projections prefetched one
chunk ahead). Startup DMAs ordered x(0),x(1), Wk, Wv, rotary tables,
Wq, attention consts so the first projections start ~4us in.
"""
import math
from contextlib import ExitStack

import numpy as np
import ml_dtypes

import concourse.bass as bass
import concourse.bacc as bacc
import concourse.tile as tile
import concourse.mybir as mybir
from concourse.bass_utils import run_bass_kernel_spmd

F32 = mybir.dt.float32
F16 = mybir.dt.float16
BF16 = mybir.dt.bfloat16
F8 = mybir.dt.float8e4
AF = mybir.ActivationFunctionType
ALU = mybir.AluOpType
DR = mybir.MatmulPerfMode.DoubleRow

B = 2
D = 2048
N_HEADS = 16
HEAD_DIM = 256
OF = 512                      # per-core q/k/v feature width (2 heads x 256)
KC2 = D // 256                # 256-deep DoubleRow contraction chunks
LAMBDA_INIT = 0.8 - 0.6 * math.exp(-0.3 * 0)
EPS = float(np.finfo(np.float32).eps)
NEG = -1.0e30
SX = 16.0                     # x pre-scale before fp8 split
SW = 128.0                    # W pre-scale before fp8 split
SV = SX * SW                  # v descale, folded into l-sum ones


_TABLES_PATCHED = False


def _patch_act_tables():
    """Force every activation into natural_log_exp_and_others so the kernel
    needs exactly one ACT table load."""
    global _TABLES_PATCHED
    if _TABLES_PATCHED:
        return
    import concourse.hw_specs as hw_specs
    mine = {AF.Exp, AF.Ln, AF.Square, AF.Copy, AF.Identity}
    orig = hw_specs.get_activation_tables

    def patched(arch):
        out = {}
        for name, funcs in orig(arch).items():
            out[name] = funcs if name == "natural_log_exp_and_others" \
                else (funcs - mine)
        return out

    bacc.get_activation_tables = patched
    _TABLES_PATCHED = True


def build_nc(T: int = 2048):
    NT = T // 128             # token tiles per batch
    NQC = T // 256            # q-chunks (token pairs) per batch
    NTB = B * NT              # token tiles total
    _patch_act_tables()
    nc = bacc.Bacc("TRN2", target_bir_lowering=False, debug=False)

    xh_d = nc.dram_tensor("xh", [128, NTB, KC2, 2, 128], F8,
                          kind="ExternalInput").ap()
    xl_d = nc.dram_tensor("xl", [128, NTB, KC2, 2, 128], F8,
                          kind="ExternalInput").ap()
    w_d = {}
    for wn in ("wqh", "wql", "wkh", "wkl", "wvh", "wvl"):
        w_d[wn] = nc.dram_tensor(wn, [128, KC2, 2, OF], F8,
                                 kind="ExternalInput").ap()
    cosd = nc.dram_tensor("cosd", [128, NT * 128], F16, kind="ExternalInput").ap()
    sind = nc.dram_tensor("sind", [128, NT * 128], F16, kind="ExternalInput").ap()
    id16d = nc.dram_tensor("id16d", [128, 128], BF16, kind="ExternalInput").ap()
    maskd = nc.dram_tensor("maskd", [128, 512], BF16, kind="ExternalInput").ap()
    onesd = nc.dram_tensor("onesd", [128, 1], BF16, kind="ExternalInput").ap()
    lamd = nc.dram_tensor("lamd", [128, 1], F32, kind="ExternalInput").ap()
    out = nc.dram_tensor("out", [B, T, OF], F32, kind="ExternalOutput").ap()

    with tile.TileContext(nc) as tc, ExitStack() as ctx:
        cpool = ctx.enter_context(tc.tile_pool(name="consts", bufs=1))
        cos_t = cpool.tile([128, NT, 128], F16, tag="cos")
        sin_t = cpool.tile([128, NT, 128], F16, tag="sin")
        id16_t = cpool.tile([128, 128], BF16, tag="id16")
        mask_t = cpool.tile([128, 512], BF16, tag="mask")
        ones_t = cpool.tile([128, 1], BF16, tag="ones")
        lam_t = cpool.tile([128, 1], F32, tag="lam")
        eps_t = cpool.tile([128, 1], F32, tag="eps")
        nc.vector.memset(eps_t[:], EPS * SV * SV)

        wpool = ctx.enter_context(tc.tile_pool(name="weights", bufs=1))
        w_t = {}
        for wn in w_d:
            w_t[wn] = wpool.tile([128, KC2, 2, OF], F8, tag=wn, name=wn)

        kvpool = ctx.enter_context(tc.tile_pool(name="kv", bufs=2))
        xpool = ctx.enter_context(tc.tile_pool(name="x", bufs=4))
        tpool = ctx.enter_context(tc.tile_pool(name="t", bufs=3))
        spool = ctx.enter_context(tc.tile_pool(name="stats", bufs=4))
        qtpool = ctx.enter_context(tc.tile_pool(name="qt", bufs=3))
        ppool = ctx.enter_context(tc.tile_pool(name="p", bufs=4))
        ypool = ctx.enter_context(tc.tile_pool(name="y", bufs=2))
        yspool = ctx.enter_context(tc.tile_pool(name="ysave", bufs=2))
        # PSUM: proj 2 + (diag-out | scores shared) 3 + out 2 + l 1 = 8 banks
        projps = ctx.enter_context(tc.tile_pool(name="projps", bufs=2, space="PSUM"))
        strps = ctx.enter_context(tc.tile_pool(name="strps", bufs=3, space="PSUM"))
        outps = ctx.enter_context(tc.tile_pool(name="outps", bufs=2, space="PSUM"))
        lps = ctx.enter_context(tc.tile_pool(name="lps", bufs=1, space="PSUM"))

        def load_w(wn):
            h = KC2 // 2
            nc.sync.dma_start(w_t[wn][:, 0:h], w_d[wn][:, 0:h])
            nc.sync.dma_start(w_t[wn][:, h:KC2], w_d[wn][:, h:KC2])

        xq = {}

        def load_x(b, tt):
            xh_t = xpool.tile([128, KC2, 2, 128], F8, tag="xh")
            xl_t = xpool.tile([128, KC2, 2, 128], F8, tag="xl")
            gt = b * NT + tt
            nc.sync.dma_start(xh_t[:], xh_d[:, gt])
            nc.sync.dma_start(xl_t[:], xl_d[:, gt])
            xq[(b, tt)] = (xh_t, xl_t)

        def get_x(b, tt):
            if (b, tt) not in xq:
                load_x(b, tt)
            return xq.pop((b, tt))

        def proj3(xts, wh, wl):
            """PSUM [128 tok, OF] = 3-term hi/lo fp8 DoubleRow projection."""
            xh_t, xl_t = xts
            ps = projps.tile([128, OF], F32, tag="proj")
            idx = 0
            for xt, wt in ((xh_t, wh), (xl_t, wh), (xh_t, wl)):
                for c in range(KC2):
                    nc.tensor.matmul(ps[:], xt[:, c], wt[:, c],
                                     start=(idx == 0), stop=(idx == 3 * KC2 - 1),
                                     perf_mode=DR)
                    idx += 1
            return ps

        def qk_process(ps, tt, dst, dst_col):
            """rms-norm + rotary + r-scaled transpose; writes bf16 d-major."""
            ps4 = ps.rearrange("p (a f) -> p a f", f=128)
            tsq = tpool.tile([128, OF], BF16, tag="tsq")
            ss = spool.tile([128, 2], F32, tag="ss")
            for h in range(2):
                nc.scalar.activation(
                    tsq[:, h * 256:(h + 1) * 256],
                    ps[:, h * 256:(h + 1) * 256],
                    AF.Square, accum_out=ss[:, h:h + 1])
            cos_b = cos_t[:, tt:tt + 1, :].broadcast_to([128, 4, 128])
            sin_b = sin_t[:, tt:tt + 1, :].broadcast_to([128, 4, 128])
            t1t = tpool.tile([128, 4, 128], BF16, tag="t1")
            t2t = tpool.tile([128, 4, 128], BF16, tag="t2")
            nc.vector.tensor_tensor(t1t[:], ps4[:], cos_b, ALU.mult)
            nc.vector.tensor_tensor(t2t[:], ps4[:], sin_b, ALU.mult)
            # r = 1/sqrt(ss/256 + eps_hat)  (scale-invariant: no descale)
            lg = spool.tile([128, 2], F32, tag="lg")
            nc.scalar.activation(lg[:], ss[:], AF.Ln, scale=1.0 / 256.0,
                                 bias=eps_t[:, 0:1])
            r = spool.tile([128, 2], F32, tag="r")
            nc.scalar.activation(r[:], lg[:], AF.Exp, scale=-0.5)
            # rotary combine -> yn [128, h, u, 128] bf16
            yn = tpool.tile([128, 2, 2, 128], BF16, tag="yn")
            t1v = t1t.rearrange("p (h u) f -> p h u f", h=2)
            t2v = t2t.rearrange("p (h u) f -> p h u f", h=2)
            nc.vector.tensor_tensor(yn[:, :, 0], t1v[:, :, 0], t2v[:, :, 1],
                                    ALU.add)
            nc.vector.tensor_tensor(yn[:, :, 1], t1v[:, :, 1], t2v[:, :, 0],
                                    ALU.subtract)
            # diag(r) per head, then transpose-with-scale via matmul
            dg = spool.tile([128, 2, 128], BF16, tag="dg")
            for h in range(2):
                nc.vector.tensor_scalar_mul(dg[:, h], id16_t[:], r[:, h:h + 1])
            tp_ps = strps.tile([128, 4, 128], F32, tag="str")
            for c in range(4):
                nc.tensor.matmul(tp_ps[:, c], yn[:, c // 2, c % 2],
                                 dg[:, c // 2], start=True, stop=True)
            nc.vector.tensor_copy(dst[:, :, dst_col:dst_col + 128], tp_ps[:])

        kv = {}

        def do_pair(b, tp):
            if tp == 0:
                kv[b] = (kvpool.tile([128, 4, T], BF16, tag="kT",
                                     name=f"kT{b}"),
                         kvpool.tile([128, NT, OF], BF16, tag="v",
                                     name=f"v{b}"))
            kT, vsb = kv[b]
            qT = qtpool.tile([128, 4, 256], BF16, tag="qT",
                             name=f"qT{b}_{tp}")
            xts2 = []
            for u2 in range(2):
                tt = 2 * tp + u2
                xts = get_x(b, tt)
                xts2.append(xts)
                kps = proj3(xts, w_t["wkh"], w_t["wkl"])
                qk_process(kps, tt, kT, tt * 128)
                vps = proj3(xts, w_t["wvh"], w_t["wvl"])
                nc.vector.tensor_copy(vsb[:, tt], vps[:])
            for u2 in range(2):
                tt = 2 * tp + u2
                qps = proj3(xts2[u2], w_t["wqh"], w_t["wql"])
                qk_process(qps, tt, qT, u2 * 128)
            return qT

        # startup DMA order: x0/x1 hi parts first, k weights, lo parts,
        # more x, v weights, rotary tables, q weights, attention consts
        xt = {}
        for tt in range(2):
            xt[tt] = (xpool.tile([128, KC2, 2, 128], F8, tag="xh",
                                 name=f"xh{tt}"),
                      xpool.tile([128, KC2, 2, 128], F8, tag="xl",
                                 name=f"xl{tt}"))
        nc.sync.dma_start(xt[0][0][:], xh_d[:, 0])
        load_w("wkh")
        nc.sync.dma_start(xt[1][0][:], xh_d[:, 1])
        load_w("wkl")
        nc.sync.dma_start(xt[0][1][:], xl_d[:, 0])
        nc.sync.dma_start(xt[1][1][:], xl_d[:, 1])
        for tt in range(2):
            xq[(0, tt)] = xt[tt]
        load_w("wvh")
        load_w("wvl")
        load_x(0, 2)
        load_x(0, 3)
        nc.sync.dma_start(cos_t[:],
                          cosd.rearrange("p (n f) -> p n f", f=128))
        nc.sync.dma_start(sin_t[:],
                          sind.rearrange("p (n f) -> p n f", f=128))
        nc.sync.dma_start(id16_t[:], id16d)
        load_w("wqh")
        load_w("wql")
        nc.sync.dma_start(mask_t[:], maskd)
        nc.sync.dma_start(ones_t[:], onesd)
        nc.sync.dma_start(lam_t[:], lamd)

        qT_next = do_pair(0, 0)
        for b in range(B):
            for qc in range(NQC):
                qT = qT_next
                if qc + 1 < NQC:
                    qT_next = do_pair(b, qc + 1)
                elif b + 1 < B:
                    qT_next = do_pair(b + 1, 0)

                kT, vsb = kv[b]
                lp2 = lps.tile([128, 2, 2, NQC], F32, tag="l")
                for s in range(2):
                    op = [outps.tile([128, OF], F32, tag="o",
                                     name=f"op{u_}") for u_ in range(2)]
                    lp = lp2[:, s]
                    for j in range(qc + 1):
                        st = strps.tile([128, 2, 256], F32, tag="str",
                                        name="sttile")
                        diag = (j == qc)
                        for t_ in range(2):
                            kt = 2 * j + t_
                            for c2 in range(2):
                                nc.tensor.matmul(
                                    st[:, t_],
                                    kT[:, 2 * s + c2, kt * 128:(kt + 1) * 128],
                                    qT[:, 2 * s + c2, :],
                                    start=(c2 == 0),
                                    stop=(c2 == 1 and not diag))
                            if diag:
                                # causal mask via PE: st += I.T @ mask
                                nc.tensor.matmul(
                                    st[:, t_], id16_t[:],
                                    mask_t[:, t_ * 256:(t_ + 1) * 256],
                                    start=False, stop=True)
                        stf = st.rearrange("p a f -> p (a f)")
                        p2 = ppool.tile([128, 2, 256], BF16, tag="p")
                        p2f = p2.rearrange("p a f -> p (a f)")
                        nc.scalar.activation(p2f, stf, AF.Exp,
                                             scale=1.0 / 16.0)
                        for u in range(2):
                            for t_ in range(2):
                                if u == 0 and j == qc and t_ == 1:
                                    continue  # fully-masked tile
                                kt = 2 * j + t_
                                lastu = 2 * qc + (1 if u == 1 else 0)
                                pu = p2[:, t_, u * 128:(u + 1) * 128]
                                nc.tensor.matmul(
                                    op[u][:], pu, vsb[:, kt],
                                    start=(kt == 0), stop=(kt == lastu))
                                nc.tensor.matmul(
                                    lp[:, u, j:j + 1], pu, ones_t[:],
                                    start=(t_ == 0),
                                    stop=(t_ == 1 or (u == 0 and j == qc)))
                    lsum = spool.tile([128, 2], F32, tag="lsum")
                    for u in range(2):
                        nc.vector.reduce_sum(lsum[:, u:u + 1],
                                             lp[:, u:u + 1, 0:qc + 1],
                                             axis=mybir.AxisListType.X)
                    linv = spool.tile([128, 2], F32, tag="linv")
                    nc.vector.reciprocal(linv[:], lsum[:])
                    if s == 1:
                        nlam = spool.tile([128, 2], F32, tag="nlam")
                        nc.vector.tensor_scalar_mul(nlam[:], linv[:],
                                                    lam_t[:, 0:1])
                        linv = nlam
                    for u in range(2):
                        if s == 0:
                            ysv = yspool.tile([128, OF], F32, tag=f"ys{u}",
                                              name=f"ys{u}")
                            nc.scalar.mul(ysv[:], op[u][:], linv[:, u:u + 1])
                            if u == 0:
                                ys0 = ysv
                            else:
                                ys1 = ysv
                        else:
                            yb = ypool.tile([128, OF], F32, tag="yb")
                            nc.scalar.mul(yb[:], op[u][:], linv[:, u:u + 1])
                            yf = ypool.tile([128, OF], F32, tag="yf")
                            nc.vector.tensor_tensor(
                                yf[:], (ys0 if u == 0 else ys1)[:], yb[:],
                                ALU.subtract)
                            t0 = qc * 256 + u * 128
                            nc.sync.dma_start(out[b, t0:t0 + 128, :], yf[:])
    nc.compile()
    return nc


def _f8_hilo(a32: np.ndarray):
    """Split f32 array into fp8e4m3 hi + residual lo (already pre-scaled)."""
    a32 = np.clip(a32, -448.0, 448.0).astype(np.float32)
    hi = a32.astype(ml_dtypes.float8_e4m3fn)
    lo = (a32 - hi.astype(np.float32)).astype(ml_dtypes.float8_e4m3fn)
    return hi, lo


def _tile_x(a8: np.ndarray, NTB: int):
    """[D, B*T] fp8 -> [128, NTB, KC2, 2, 128] per-tile layout."""
    return np.ascontiguousarray(
        a8.reshape(KC2, 2, 128, NTB, 128).transpose(2, 3, 0, 1, 4))


def _tile_w(w8: np.ndarray):
    """[D, OF] fp8 -> [128, KC2, 2, OF]."""
    return np.ascontiguousarray(
        w8.reshape(KC2, 2, 128, OF).transpose(2, 0, 1, 3))


def make_in_maps(x, Wq, Wk, Wv, lam, T):
    NT = T // 128
    NTB = B * NT
    xf = np.ascontiguousarray(x.reshape(B * T, D).T)          # [D, B*T]
    xh8, xl8 = _f8_hilo(xf * SX)
    xh_t = _tile_x(xh8, NTB)
    xl_t = _tile_x(xl8, NTB)

    t = np.arange(T, dtype=np.float64)
    inv = 1.0 / (10000.0 ** (np.arange(0, HEAD_DIM, 2, dtype=np.float64)
                             / HEAD_DIM))
    fr = np.outer(t, inv)
    cos = np.cos(fr).astype(np.float32)
    sin = np.sin(fr).astype(np.float32)
    cos_sb = np.ascontiguousarray(
        cos.reshape(NT, 128, 128).transpose(1, 0, 2).reshape(128, NT * 128)
    ).astype(np.float16)
    sin_sb = np.ascontiguousarray(
        sin.reshape(NT, 128, 128).transpose(1, 0, 2).reshape(128, NT * 128)
    ).astype(np.float16)

    id16 = np.eye(128).astype(ml_dtypes.bfloat16)
    kk = np.arange(128).reshape(128, 1)
    qq = np.arange(128).reshape(1, 128)
    tri = np.where(qq >= kk, 0.0, NEG).astype(np.float32)
    zeros = np.zeros((128, 128), np.float32)
    negs = np.full((128, 128), NEG, np.float32)
    maskp = np.concatenate([tri, zeros, negs, tri], axis=1).astype(
        ml_dtypes.bfloat16)                                  # [128, 512]
    ones2048 = np.full((128, 1), SV, ml_dtypes.bfloat16)
    lam_np = np.full((128, 1), lam, np.float32)

    common = {"xh": xh_t, "xl": xl_t, "cosd": cos_sb, "sind": sin_sb,
              "id16d": id16, "maskd": maskp, "onesd": ones2048,
              "lamd": lam_np}
    in_maps = []
    for i in range(8):
        m = dict(common)
        for nm, W, rows in (("wq", Wq, None), ("wk", Wk, None),
                            ("wv", Wv, i)):
            if rows is None:
                w_sh = np.concatenate(
                    [W[i * 256:(i + 1) * 256],
                     W[(i + 8) * 256:(i + 9) * 256]], 0)
            else:
                w_sh = W[i * 512:(i + 1) * 512]
            wh8, wl8 = _f8_hilo(np.ascontiguousarray(w_sh.T) * SW)
            m[nm + "h"] = _tile_w(wh8)
            m[nm + "l"] = _tile_w(wl8)
        in_maps.append(m)
    return in_maps


_NC_CACHE: dict = {}


def run_cores(x, Wq, Wk, Wv, lambda_q1, lambda_k1, lambda_q2, lambda_k2,
              T=2048, **spmd_kwargs):
    lam1 = np.exp(np.float32(np.dot(lambda_q1.astype(np.float32),
                                    lambda_k1.astype(np.float32))))
    lam2 = np.exp(np.float32(np.dot(lambda_q2.astype(np.float32),
                                    lambda_k2.astype(np.float32))))
    lam = np.float32(lam1 - lam2 + np.float32(LAMBDA_INIT))
    if T not in _NC_CACHE:
        _NC_CACHE[T] = build_nc(T)
    nc = _NC_CACHE[T]
    in_maps = make_in_maps(np.asarray(x), np.asarray(Wq), np.asarray(Wk),
                           np.asarray(Wv), lam, T)
    res = run_bass_kernel_spmd(nc, in_maps, core_ids=list(range(8)),
                               **spmd_kwargs)
    shards = [res.results[i]["out"] for i in range(8)]       # [B,T,512] each
    y = np.stack(shards, axis=2).reshape(B, x.shape[1], N_HEADS * HEAD_DIM)
    return y, res


def kernel(x, Wq, Wk, Wv, lambda_q1, lambda_k1, lambda_q2, lambda_k2):
    y, _ = run_cores(x, Wq, Wk, Wv, lambda_q1, lambda_k1, lambda_q2,
                     lambda_k2, T=x.shape[1])
    return y.astype(np.float32)


# revision 3
# speedup vs baseline: 1.0000x; 1.0000x over previous
"""MixerDiffAttention TRN2 kernel v2 (8-core TP over head pairs).

Per-core math (head-pair i: q/k heads i,i+8; v head i):
  q,k,v projections as fp8e4m3 DoubleRow matmuls with hi/lo error
  compensation: host splits x*16 and W*128 into fp8 hi + fp8 residual;
  q = xh@Wh + xl@Wh + xh@Wl (0.75x fp32r cost, ~1e-3 accurate).
  rms_norm is scale-invariant so q/k need no descale; the v descale
  (1/2048) is folded into the l-sum ones constant (2048.0).
  Scores/PV/p in bf16. Per-head rms normalization folds into the
  transpose: one bf16 matmul against diag(r) built by DVE from a bf16
  identity (qT = yn.T @ diag(r)). exp applies the 1/sqrt(256) scale.

Schedule: single pass per batch - each token pair projects k/v/q then
runs attention for that q-chunk (next pair's projections prefetched one
chunk ahead). Startup DMAs ordered x(0),x(1), Wk, Wv, rotary tables,
Wq, attention consts so the first projections start ~4us in.
"""
import math
from contextlib import ExitStack

import numpy as np
import ml_dtypes

import concourse.bass as bass
import concourse.bacc as bacc
import concourse.tile as tile
import concourse.mybir as mybir
from concourse.bass_utils import run_bass_kernel_spmd

F32 = mybir.dt.float32
F16 = mybir.dt.float16
BF16 = mybir.dt.bfloat16
F8 = mybir.dt.float8e4
AF = mybir.ActivationFunctionType
ALU = mybir.AluOpType
DR = mybir.MatmulPerfMode.DoubleRow

B = 2
D = 2048
N_HEADS = 16
HEAD_DIM = 256
OF = 512                      # per-core q/k/v feature width (2 heads x 256)
KC2 = D // 256                # 256-deep DoubleRow contraction chunks
LAMBDA_INIT = 0.8 - 0.6 * math.exp(-0.3 * 0)
EPS = float(np.finfo(np.float32).eps)
NEG = -1.0e30
SX = 16.0                     # x pre-scale before fp8 split
SW = 128.0                    # W pre-scale before fp8 split
SV = SX * SW                  # v descale, folded into l-sum ones


_TABLES_PATCHED = False


def _patch_act_tables():
    """Force every activation into natural_log_exp_and_others so the kernel
    needs exactly one ACT table load."""
    global _TABLES_PATCHED
    if _TABLES_PATCHED:
        return
    import concourse.hw_specs as hw_specs
    mine = {AF.Exp, AF.Ln, AF.Square, AF.Copy, AF.Identity}
    orig = hw_specs.get_activation_tables

    def patched(arch):
        out = {}
        for name, funcs in orig(arch).items():
            out[name] = funcs if name == "natural_log_exp_and_others" \
                else (funcs - mine)
        return out

    bacc.get_activation_tables = patched
    _TABLES_PATCHED = True


def build_nc(T: int = 2048):
    NT = T // 128             # token tiles per batch
    NQC = T // 256            # q-chunks (token pairs) per batch
    NTB = B * NT              # token tiles total
    _patch_act_tables()
    nc = bacc.Bacc("TRN2", target_bir_lowering=False, debug=False)

    xh_d = nc.dram_tensor("xh", [128, NTB, KC2, 2, 128], F8,
                          kind="ExternalInput").ap()
    xl_d = nc.dram_tensor("xl", [128, NTB, KC2, 2, 128], F8,
                          kind="ExternalInput").ap()
    w_d = {}
    for wn in ("wqh", "wql", "wkh", "wkl", "wvh", "wvl"):
        w_d[wn] = nc.dram_tensor(wn, [128, KC2, 2, OF], F8,
                                 kind="ExternalInput").ap()
    cosd = nc.dram_tensor("cosd", [128, NT * 128], F16, kind="ExternalInput").ap()
    sind = nc.dram_tensor("sind", [128, NT * 128], F16, kind="ExternalInput").ap()
    id16d = nc.dram_tensor("id16d", [128, 128], BF16, kind="ExternalInput").ap()
    maskd = nc.dram_tensor("maskd", [128, 512], F32, kind="ExternalInput").ap()
    onesd = nc.dram_tensor("onesd", [128, 1], BF16, kind="ExternalInput").ap()
    lamd = nc.dram_tensor("lamd", [128, 1], F32, kind="ExternalInput").ap()
    out = nc.dram_tensor("out", [B, T, OF], F32, kind="ExternalOutput").ap()

    with tile.TileContext(nc) as tc, ExitStack() as ctx:
        cpool = ctx.enter_context(tc.tile_pool(name="consts", bufs=1))
        cos_t = cpool.tile([128, NT, 128], F16, tag="cos")
        sin_t = cpool.tile([128, NT, 128], F16, tag="sin")
        id16_t = cpool.tile([128, 128], BF16, tag="id16")
        mask_t = cpool.tile([128, 512], F32, tag="mask")
        ones_t = cpool.tile([128, 1], BF16, tag="ones")
        lam_t = cpool.tile([128, 1], F32, tag="lam")
        eps_t = cpool.tile([128, 1], F32, tag="eps")
        nc.vector.memset(eps_t[:], EPS * SV * SV)

        wpool = ctx.enter_context(tc.tile_pool(name="weights", bufs=1))
        w_t = {}
        for wn in w_d:
            w_t[wn] = wpool.tile([128, KC2, 2, OF], F8, tag=wn, name=wn)

        kvpool = ctx.enter_context(tc.tile_pool(name="kv", bufs=2))
        xpool = ctx.enter_context(tc.tile_pool(name="x", bufs=4))
        tpool = ctx.enter_context(tc.tile_pool(name="t", bufs=3))
        spool = ctx.enter_context(tc.tile_pool(name="stats", bufs=4))
        qtpool = ctx.enter_context(tc.tile_pool(name="qt", bufs=3))
        ppool = ctx.enter_context(tc.tile_pool(name="p", bufs=4))
        ypool = ctx.enter_context(tc.tile_pool(name="y", bufs=2))
        yspool = ctx.enter_context(tc.tile_pool(name="ysave", bufs=2))
        # PSUM: proj 2 + (diag-out | scores shared) 3 + out 2 + l 1 = 8 banks
        projps = ctx.enter_context(tc.tile_pool(name="projps", bufs=2, space="PSUM"))
        strps = ctx.enter_context(tc.tile_pool(name="strps", bufs=3, space="PSUM"))
        outps = ctx.enter_context(tc.tile_pool(name="outps", bufs=2, space="PSUM"))
        lps = ctx.enter_context(tc.tile_pool(name="lps", bufs=1, space="PSUM"))

        def load_w(wn):
            h = KC2 // 2
            nc.sync.dma_start(w_t[wn][:, 0:h], w_d[wn][:, 0:h])
            nc.sync.dma_start(w_t[wn][:, h:KC2], w_d[wn][:, h:KC2])

        xq = {}

        def load_x(b, tt):
            xh_t = xpool.tile([128, KC2, 2, 128], F8, tag="xh")
            xl_t = xpool.tile([128, KC2, 2, 128], F8, tag="xl")
            gt = b * NT + tt
            nc.sync.dma_start(xh_t[:], xh_d[:, gt])
            nc.sync.dma_start(xl_t[:], xl_d[:, gt])
            xq[(b, tt)] = (xh_t, xl_t)

        def get_x(b, tt):
            if (b, tt) not in xq:
                load_x(b, tt)
            return xq.pop((b, tt))

        def proj3(xts, wh, wl):
            """PSUM [128 tok, OF] = 3-term hi/lo fp8 DoubleRow projection."""
            xh_t, xl_t = xts
            ps = projps.tile([128, OF], F32, tag="proj")
            idx = 0
            for xt, wt in ((xh_t, wh), (xl_t, wh), (xh_t, wl)):
                for c in range(KC2):
                    nc.tensor.matmul(ps[:], xt[:, c], wt[:, c],
                                     start=(idx == 0), stop=(idx == 3 * KC2 - 1),
                                     perf_mode=DR)
                    idx += 1
            return ps

        def qk_process(ps, tt, dst, dst_col):
            """rms-norm + rotary + r-scaled transpose; writes bf16 d-major."""
            ps4 = ps.rearrange("p (a f) -> p a f", f=128)
            tsq = tpool.tile([128, OF], BF16, tag="tsq")
            ss = spool.tile([128, 2], F32, tag="ss")
            for h in range(2):
                nc.scalar.activation(
                    tsq[:, h * 256:(h + 1) * 256],
                    ps[:, h * 256:(h + 1) * 256],
                    AF.Square, accum_out=ss[:, h:h + 1])
            cos_b = cos_t[:, tt:tt + 1, :].broadcast_to([128, 4, 128])
            sin_b = sin_t[:, tt:tt + 1, :].broadcast_to([128, 4, 128])
            t1t = tpool.tile([128, 4, 128], BF16, tag="t1")
            t2t = tpool.tile([128, 4, 128], BF16, tag="t2")
            nc.vector.tensor_tensor(t1t[:], ps4[:], cos_b, ALU.mult)
            nc.vector.tensor_tensor(t2t[:], ps4[:], sin_b, ALU.mult)
            # r = 1/sqrt(ss/256 + eps_hat)  (scale-invariant: no descale)
            lg = spool.tile([128, 2], F32, tag="lg")
            nc.scalar.activation(lg[:], ss[:], AF.Ln, scale=1.0 / 256.0,
                                 bias=eps_t[:, 0:1])
            r = spool.tile([128, 2], F32, tag="r")
            nc.scalar.activation(r[:], lg[:], AF.Exp, scale=-0.5)
            # rotary combine -> yn [128, h, u, 128] bf16
            yn = tpool.tile([128, 2, 2, 128], BF16, tag="yn")
            t1v = t1t.rearrange("p (h u) f -> p h u f", h=2)
            t2v = t2t.rearrange("p (h u) f -> p h u f", h=2)
            nc.vector.tensor_tensor(yn[:, :, 0], t1v[:, :, 0], t2v[:, :, 1],
                                    ALU.add)
            nc.vector.tensor_tensor(yn[:, :, 1], t1v[:, :, 1], t2v[:, :, 0],
                                    ALU.subtract)
            # diag(r) per head, then transpose-with-scale via matmul
            dg = spool.tile([128, 2, 128], BF16, tag="dg")
            for h in range(2):
                nc.vector.tensor_scalar_mul(dg[:, h], id16_t[:], r[:, h:h + 1])
            tp_ps = strps.tile([128, 4, 128], F32, tag="str")
            for c in range(4):
                nc.tensor.matmul(tp_ps[:, c], yn[:, c // 2, c % 2],
                                 dg[:, c // 2], start=True, stop=True)
            nc.vector.tensor_copy(dst[:, :, dst_col:dst_col + 128], tp_ps[:])

        kv = {}

        def do_pair(b, tp):
            if tp == 0:
                kv[b] = (kvpool.tile([128, 4, T], BF16, tag="kT",
                                     name=f"kT{b}"),
                         kvpool.tile([128, NT, OF], BF16, tag="v",
                                     name=f"v{b}"))
            kT, vsb = kv[b]
            qT = qtpool.tile([128, 4, 256], BF16, tag="qT",
                             name=f"qT{b}_{tp}")
            xts2 = []
            for u2 in range(2):
                tt = 2 * tp + u2
                xts = get_x(b, tt)
                xts2.append(xts)
                kps = proj3(xts, w_t["wkh"], w_t["wkl"])
                qk_process(kps, tt, kT, tt * 128)
                vps = proj3(xts, w_t["wvh"], w_t["wvl"])
                nc.vector.tensor_copy(vsb[:, tt], vps[:])
            for u2 in range(2):
                tt = 2 * tp + u2
                qps = proj3(xts2[u2], w_t["wqh"], w_t["wql"])
                qk_process(qps, tt, qT, u2 * 128)
            return qT

        # startup DMA order: x0/x1 hi parts first, k weights, lo parts,
        # more x, v weights, rotary tables, q weights, attention consts
        xt = {}
        for tt in range(2):
            xt[tt] = (xpool.tile([128, KC2, 2, 128], F8, tag="xh",
                                 name=f"xh{tt}"),
                      xpool.tile([128, KC2, 2, 128], F8, tag="xl",
                                 name=f"xl{tt}"))
        nc.sync.dma_start(xt[0][0][:], xh_d[:, 0])
        load_w("wkh")
        nc.sync.dma_start(xt[1][0][:], xh_d[:, 1])
        load_w("wkl")
        nc.sync.dma_start(xt[0][1][:], xl_d[:, 0])
        nc.sync.dma_start(xt[1][1][:], xl_d[:, 1])
        for tt in range(2):
            xq[(0, tt)] = xt[tt]
        load_w("wvh")
        load_w("wvl")
        load_x(0, 2)
        load_x(0, 3)
        nc.sync.dma_start(cos_t[:],
                          cosd.rearrange("p (n f) -> p n f", f=128))
        nc.sync.dma_start(sin_t[:],
                          sind.rearrange("p (n f) -> p n f", f=128))
        nc.sync.dma_start(id16_t[:], id16d)
        load_w("wqh")
        load_w("wql")
        nc.sync.dma_start(mask_t[:], maskd)
        nc.sync.dma_start(ones_t[:], onesd)
        nc.sync.dma_start(lam_t[:], lamd)

        qT_next = do_pair(0, 0)
        for b in range(B):
            for qc in range(NQC):
                qT = qT_next
                if qc + 1 < NQC:
                    qT_next = do_pair(b, qc + 1)
                elif b + 1 < B:
                    qT_next = do_pair(b + 1, 0)

                kT, vsb = kv[b]
                lp2 = lps.tile([128, 2, 2, NQC], F32, tag="l")
                for s in range(2):
                    op = [outps.tile([128, OF], F32, tag="o",
                                     name=f"op{u_}") for u_ in range(2)]
                    lp = lp2[:, s]
                    for j in range(qc + 1):
                        st = strps.tile([128, 2, 256], F32, tag="str",
                                        name="sttile")
                        for t_ in range(2):
                            kt = 2 * j + t_
                            for c2 in range(2):
                                nc.tensor.matmul(
                                    st[:, t_],
                                    kT[:, 2 * s + c2, kt * 128:(kt + 1) * 128],
                                    qT[:, 2 * s + c2, :],
                                    start=(c2 == 0), stop=(c2 == 1))
                        stf = st.rearrange("p a f -> p (a f)")
                        if j == qc:
                            nc.vector.tensor_tensor(stf, stf, mask_t[:],
                                                    ALU.add)
                        p2 = ppool.tile([128, 2, 256], BF16, tag="p")
                        p2f = p2.rearrange("p a f -> p (a f)")
                        nc.scalar.activation(p2f, stf, AF.Exp,
                                             scale=1.0 / 16.0)
                        for u in range(2):
                            for t_ in range(2):
                                if u == 0 and j == qc and t_ == 1:
                                    continue  # fully-masked tile
                                kt = 2 * j + t_
                                lastu = 2 * qc + (1 if u == 1 else 0)
                                pu = p2[:, t_, u * 128:(u + 1) * 128]
                                nc.tensor.matmul(
                                    op[u][:], pu, vsb[:, kt],
                                    start=(kt == 0), stop=(kt == lastu))
                                nc.tensor.matmul(
                                    lp[:, u, j:j + 1], pu, ones_t[:],
                                    start=(t_ == 0),
                                    stop=(t_ == 1 or (u == 0 and j == qc)))
                    lsum = spool.tile([128, 2], F32, tag="lsum")
                    for u in range(2):
                        nc.vector.reduce_sum(lsum[:, u:u + 1],
                                             lp[:, u:u + 1, 0:qc + 1],
                                             axis=mybir.AxisListType.X)
                    linv = spool.tile([128, 2], F32, tag="linv")
                    nc.vector.reciprocal(linv[:], lsum[:])
                    if s == 1:
                        nlam = spool.tile([128, 2], F32, tag="nlam")
                        nc.vector.tensor_scalar_mul(nlam[:], linv[:],
                                                    lam_t[:, 0:1])
                        linv = nlam
                    for u in range(2):
                        if s == 0:
                            ysv = yspool.tile([128, OF], F32, tag=f"ys{u}",
                                              name=f"ys{u}")
                            nc.scalar.mul(ysv[:], op[u][:], linv[:, u:u + 1])
                            if u == 0:
                                ys0 = ysv
                            else:
                                ys1 = ysv
                        else:
                            yb = ypool.tile([128, OF], F32, tag="yb")
                            nc.scalar.mul(yb[:], op[u][:], linv[:, u:u + 1])
                            yf = ypool.tile([128, OF], F32, tag="yf")
                            nc.vector.tensor_tensor(
                                yf[:], (ys0 if u == 0 else ys1)[:], yb[:],
                                ALU.subtract)
                            t0 = qc * 256 + u * 128
                            nc.sync.dma_start(out[b, t0:t0 + 128, :], yf[:])
    nc.compile()
    return nc


def _f8_hilo(a32: np.ndarray):
    """Split f32 array into fp8e4m3 hi + residual lo (already pre-scaled)."""
    a32 = np.clip(a32, -448.0, 448.0).astype(np.float32)
    hi = a32.astype(ml_dtypes.float8_e4m3fn)
    lo = (a32 - hi.astype(np.float32)).astype(ml_dtypes.float8_e4m3fn)
    return hi, lo


def _tile_x(a8: np.ndarray, NTB: int):
    """[D, B*T] fp8 -> [128, NTB, KC2, 2, 128] per-tile layout."""
    return np.ascontiguousarray(
        a8.reshape(KC2, 2, 128, NTB, 128).transpose(2, 3, 0, 1, 4))


def _tile_w(w8: np.ndarray):
    """[D, OF] fp8 -> [128, KC2, 2, OF]."""
    return np.ascontiguousarray(
        w8.reshape(KC2, 2, 128, OF).transpose(2, 0, 1, 3))


def make_in_maps(x, Wq, Wk, Wv, lam, T):
    NT = T // 128
    NTB = B * NT
    xf = np.ascontiguousarray(x.reshape(B * T, D).T)          # [D, B*T]
    xh8, xl8 = _f8_hilo(xf * SX)
    xh_t = _tile_x(xh8, NTB)
    xl_t = _tile_x(xl8, NTB)

    t = np.arange(T, dtype=np.float64)
    inv = 1.0 / (10000.0 ** (np.arange(0, HEAD_DIM, 2, dtype=np.float64)
                             / HEAD_DIM))
    fr = np.outer(t, inv)
    cos = np.cos(fr).astype(np.float32)
    sin = np.sin(fr).astype(np.float32)
    cos_sb = np.ascontiguousarray(
        cos.reshape(NT, 128, 128).transpose(1, 0, 2).reshape(128, NT * 128)
    ).astype(np.float16)
    sin_sb = np.ascontiguousarray(
        sin.reshape(NT, 128, 128).transpose(1, 0, 2).reshape(128, NT * 128)
    ).astype(np.float16)

    id16 = np.eye(128).astype(ml_dtypes.bfloat16)
    kk = np.arange(128).reshape(128, 1)
    qq = np.arange(128).reshape(1, 128)
    tri = np.where(qq >= kk, 0.0, NEG).astype(np.float32)
    zeros = np.zeros((128, 128), np.float32)
    negs = np.full((128, 128), NEG, np.float32)
    maskp = np.concatenate([tri, zeros, negs, tri], axis=1)   # [128, 512]
    ones2048 = np.full((128, 1), SV, ml_dtypes.bfloat16)
    lam_np = np.full((128, 1), lam, np.float32)

    common = {"xh": xh_t, "xl": xl_t, "cosd": cos_sb, "sind": sin_sb,
              "id16d": id16, "maskd": maskp, "onesd": ones2048,
              "lamd": lam_np}
    in_maps = []
    for i in range(8):
        m = dict(common)
        for nm, W, rows in (("wq", Wq, None), ("wk", Wk, None),
                            ("wv", Wv, i)):
            if rows is None:
                w_sh = np.concatenate(
                    [W[i * 256:(i + 1) * 256],
                     W[(i + 8) * 256:(i + 9) * 256]], 0)
            else:
                w_sh = W[i * 512:(i + 1) * 512]
            wh8, wl8 = _f8_hilo(np.ascontiguousarray(w_sh.T) * SW)
            m[nm + "h"] = _tile_w(wh8)
            m[nm + "l"] = _tile_w(wl8)
        in_maps.append(m)
    return in_maps


_NC_CACHE: dict = {}


def run_cores(x, Wq, Wk, Wv, lambda_q1, lambda_k1, lambda_q2, lambda_k2,
              T=2048, **spmd_kwargs):
    lam1 = np.exp(np.float32(np.dot(lambda_q1.astype(np.float32),
                                    lambda_k1.astype(np.float32))))
    lam2 = np.exp(np.float32(np.dot(lambda_q2.astype(np.float32),
                                    lambda_k2.astype(np.float32))))
    lam = np.float32(lam1 - lam2 + np.float32(LAMBDA_INIT))
    if T not in _NC_CACHE:
        _NC_CACHE[T] = build_nc(T)
    nc = _NC_CACHE[T]
    in_maps = make_in_maps(np.asarray(x), np.asarray(Wq), np.asarray(Wk),
                           np.asarray(Wv), lam, T)
    res = run_bass_kernel_spmd(nc, in_maps, core_ids=list(range(8)),
                               **spmd_kwargs)
    shards = [res.results[i]["out"] for i in range(8)]       # [B,T,512] each
    y = np.stack(shards, axis=2).reshape(B, x.shape[1], N_HEADS * HEAD_DIM)
    return y, res


def kernel(x, Wq, Wk, Wv, lambda_q1, lambda_k1, lambda_q2, lambda_k2):
    y, _ = run_cores(x, Wq, Wk, Wv, lambda_q1, lambda_k1, lambda_q2,
                     lambda_k2, T=x.shape[1])
    return y.astype(np.float32)
